# revision 1
# baseline (speedup 1.0000x reference)
"""
Trainium2 Bass kernel for EnhancedIsotropyMaximizationLoss.

loss = 1/diag_var_mean + log(mean(pairwise_L2_distance(c)))
where c = row-L2-normalized embeddings, centered by the column mean.

Key algebra:
  * pairwise distances are translation invariant, so the centering is NOT
    applied on device -- distances use the normalized (uncentered) rows e,
    and the variance term is corrected on host:
      sum(c*c) = sum(e*e) - N*||mu||^2,  mu = mean(e, axis=0).
  * per-row s_i = ||e_i||^2 = (||x_i|| / (||x_i|| + eps))^2 comes straight
    from the row-space norms (no transposed reduction needed).

Distribution (8 cores, no collectives): circulant decomposition of the
64x64 grid of 128-row tiles; core k gets the input rotated by k*1024 rows
and computes, for each local row-tile m in 0..7: diag tile (w=1),
d=1..31 (w=2), d=32 (w=1).  Column tiles beyond m+32 <= 39 are never
touched, so only the first 40 row-tiles (5 MB of 8 MB) are loaded,
normalized and transposed per core.  Partial sums combined on host.

On-device pipeline per core (engine-balanced, single PSUM lifetime and
per-4-tile chains so the load/normalize/transpose stream overlaps the
distance blocks):
  DMA      x[5120,256] f32 in 10 transfers
  DVE      bn_stats row norms + r chain, half the scales, drains of
           groups 1-2, s/bias copies, esum
  GpSimd   half the scales, all of sq2sum (ct0^2, ct1^2, add)
  ScalarE  group-0 drains (front window), then every distance
           sqrt+accumulate: PSUM holds g - s_j/2 in f32 from an fp8
           DoubleRow gram pass + a -1/2-ones s_j pass, and the
           activation applies sqrt(-2*psum + s_i + guard)
  PE       bf16 transposes, fp8 DoubleRow gram, s_j passes, s_i matmuls
"""

import sys

if "/opt/trn_rl_repo" not in sys.path:
    sys.path.insert(0, "/opt/trn_rl_repo")

import numpy as np

N, D, P = 8192, 256, 128
NT = N // P          # 64 row tiles
NCORES = 8
LT = NT // NCORES    # 8 local row tiles per core
NTU = LT + 32        # 40 row tiles actually used per core
NU = NTU * P         # 5120 rows loaded per core
EPS = 1e-6
# positivity guards under the sqrt; bf16 storage of sq2sum vs f32 PSUM gram
# can mismatch the diagonal by ~0.5% of s (~0.01), so the diag guard must
# cover that.  Off-diagonal sq_dists are >= ~1.3 for N(0,1) data.
DELTA_BG = 2e-2
DELTA_SM = 1e-4

_CACHE = {}


def _build(stage=5, reps=1):
    import concourse.bacc as bacc
    import concourse.tile as tile
    from concourse import masks, mybir

    Op = mybir.AluOpType
    Act = mybir.ActivationFunctionType
    F32 = mybir.dt.float32
    F32R = mybir.dt.float32r
    BF16 = mybir.dt.bfloat16
    FP8 = mybir.dt.float8e4
    AX = mybir.AxisListType

    nc = bacc.Bacc("TRN2", target_bir_lowering=False, debug=False)
    x = nc.dram_tensor("x", [NU, D], F32, kind="ExternalInput").ap()
    out = nc.dram_tensor("out", [P, 8], F32, kind="ExternalOutput").ap()

    with tile.TileContext(nc) as tc:
        from contextlib import ExitStack

        ctx = ExitStack()
        with ctx:
            singles = ctx.enter_context(tc.tile_pool(name="singles", bufs=1))
            ct = singles.tile([P, 2, NU], FP8, tag="ct")
            sq2 = singles.tile([P, NU], BF16, tag="sq2")
            sqt = singles.tile([P, NU], BF16, tag="sqt")
            onesb = singles.tile([P, 3, P], BF16, tag="onesb")
            identb = singles.tile([P, P], BF16, tag="identb")
            s_loc = singles.tile([P, LT], F32, tag="s_loc")
            bias_sm = singles.tile([P, LT], F32, tag="bias_sm")
            bias_bg = singles.tile([P, LT], F32, tag="bias_bg")
            acc_w1 = singles.tile([P, LT], F32, tag="acc_w1")
            acc_w2 = singles.tile([P, 3 * LT], F32, tag="acc_w2")
            esum16 = singles.tile([P, 2, 8], F32, tag="esum16")
            esum = singles.tile([P, 2], F32, tag="esum")
            out_sb = singles.tile([P, 8], F32, tag="out_sb")

            onescol = singles.tile([P, 1], BF16, tag="onescol")
            nc.vector.memset(onescol[:], 1.0)
            mhcol = singles.tile([P, 1], BF16, tag="mhcol")
            nc.vector.memset(mhcol[:], -0.5)
            for cp in range(3):
                nc.vector.tensor_copy(onesb[:, cp, :],
                                      mhcol[:].to_broadcast([P, P]))
            masks.make_identity(nc, identb[:])
            # preload the sqrt act table while everything else warms up
            warm = singles.tile([P, 1], F32, tag="warm")
            nc.scalar.activation(warm[:], onescol[:], Act.Sqrt)
            if stage != 5:
                for tl in (s_loc, bias_sm, bias_bg, acc_w1, acc_w2):
                    nc.vector.memset(tl[:], 0.5)
                nc.vector.memset(esum[:], 0.0)
                nc.vector.memset(esum16[:], 0.0)

            for _rep in range(reps):
                GROUPS = [(0, 16), (16, 16), (32, 8)]
                with (
                    tc.tile_pool(name="xa", bufs=9) as xa_pool,
                    tc.tile_pool(name="ssg", bufs=3) as ssg_pool,
                    tc.tile_pool(name="dgr", bufs=8) as dg_pool,
                    tc.tile_pool(name="ptp", bufs=1, space="PSUM") as pt_pool,
                    tc.tile_pool(name="blk", bufs=2, space="PSUM") as blk_pool,
                    tc.tile_pool(name="trs", bufs=3) as tr_pool,
                    tc.tile_pool(name="trd", bufs=2) as trd_pool,
                ):
                    ct0v = ct[:, 0, :].rearrange("p (t c) -> p t c", c=P)
                    ct1v = ct[:, 1, :].rearrange("p (t c) -> p t c", c=P)
                    sq2v = sq2[:].rearrange("p (t c) -> p t c", c=P)
                    wrot = [0]

                    def group(gt0, gn, upto=3):
                        """Pipelined chains: DMA -> bn_stats -> sqrt/recip ->
                        scale (DVE/GpSimd) -> PE transpose -> drain.  Group 0
                        runs per-4-tile chunks (latency-critical); later
                        groups batch two chunks per sqrt/recip to cut
                        instruction count."""
                        nrm_g = ssg_pool.tile([P, 16], F32, tag="nrm")
                        den_g = ssg_pool.tile([P, 16], F32, tag="den")
                        r_g = ssg_pool.tile([P, 16], F32, tag="rg")
                        bng = ssg_pool.tile([P, 16, 6], F32, tag="bng")
                        ta_g = ssg_pool.tile([P, 16], F32, tag="ta")
                        tb_g = ssg_pool.tile([P, 16], F32, tag="tb")

                        def front(qq):
                            """DMA + bn_stats + sumsq post-math for chunk qq."""
                            t0 = gt0 + qq * 4
                            xb = xa_pool.tile([P, 4, D], F32, tag="xt")
                            dma_eng = nc.sync if qq % 2 == 0 else nc.scalar
                            dma_eng.dma_start(
                                out=xb[:],
                                in_=x[t0 * P:(t0 + 4) * P, :].rearrange(
                                    "(a p) d -> p a d", p=P))
                            for j in range(4):
                                q = qq * 4 + j
                                nc.vector.bn_stats(bng[:, q, :], xb[:, j, :])
                            q0, q1 = qq * 4, qq * 4 + 4
                            csl = (slice(None), slice(q0, q1))
                            # sumsq = M2_e + M2_o + 128*(mean_e^2 + mean_o^2)
                            # (post-math on GpSimd; DVE is the pacer)
                            nc.gpsimd.tensor_tensor(
                                out=ta_g[csl], in0=bng[:, q0:q1, 1],
                                in1=bng[:, q0:q1, 1], op=Op.mult)
                            nc.gpsimd.tensor_tensor(
                                out=tb_g[csl], in0=bng[:, q0:q1, 4],
                                in1=bng[:, q0:q1, 4], op=Op.mult)
                            nc.gpsimd.tensor_tensor(
                                out=ta_g[csl], in0=ta_g[csl], in1=tb_g[csl],
                                op=Op.add)
                            nc.gpsimd.tensor_scalar(
                                out=ta_g[csl], in0=ta_g[csl],
                                scalar1=float(D // 2), scalar2=None,
                                op0=Op.mult)
                            nc.gpsimd.tensor_tensor(
                                out=nrm_g[csl], in0=bng[:, q0:q1, 2],
                                in1=bng[:, q0:q1, 5], op=Op.add)
                            nc.gpsimd.tensor_tensor(
                                out=nrm_g[csl], in0=nrm_g[csl], in1=ta_g[csl],
                                op=Op.add)
                            return xb

                        def rchain(b0, b1):
                            """nrm = ||x||, r = 1/(nrm+EPS) over cols [b0,b1)."""
                            bsl = (slice(None), slice(b0, b1))
                            nc.scalar.activation(nrm_g[bsl], nrm_g[bsl],
                                                 Act.Sqrt)
                            nc.gpsimd.tensor_scalar(
                                out=den_g[bsl], in0=nrm_g[bsl], scalar1=EPS,
                                scalar2=None, op0=Op.add)
                            nc.vector.reciprocal(r_g[bsl], den_g[bsl])

                        def back(qq, xb):
                            """scales, transposes, drain for chunk qq."""
                            t0 = gt0 + qq * 4
                            pt = pt_pool.tile([P, 2, 4, P], BF16, tag="pt")
                            for j in range(4):
                                q = qq * 4 + j
                                et = dg_pool.tile([P, D], BF16, tag="et")
                                # e = x * r -> fp8, split DVE/GpSimd; drains
                                # live on DVE (GpSimd has no PSUM port).
                                seng = nc.vector if j % 2 == 0 else nc.gpsimd
                                seng.tensor_scalar(
                                    out=et[:], in0=xb[:, j, :],
                                    scalar1=r_g[:, q:q + 1], scalar2=None,
                                    op0=Op.mult)
                                if upto < 2:
                                    continue
                                nc.tensor.transpose(
                                    pt[:, 0, j, :], et[:, 0:P], identb[:])
                                nc.tensor.transpose(
                                    pt[:, 1, j, :], et[:, P:2 * P], identb[:])
                            if upto >= 3:
                                c0 = t0 * P
                                sl = slice(c0, c0 + 4 * P)
                                # group-0 drains ride ScalarE's idle front
                                # window; later groups drain on DVE
                                if gt0 < 16:
                                    nc.scalar.copy(
                                        ct[:, :, sl],
                                        pt[:].rearrange("p h a d -> p h (a d)"))
                                else:
                                    nc.vector.tensor_copy(
                                        ct[:, :, sl],
                                        pt[:].rearrange("p h a d -> p h (a d)"))

                        if gt0 == 0:
                            for qq in range(gn // 4):
                                xb = front(qq)
                                rchain(qq * 4, qq * 4 + 4)
                                if upto >= 1:
                                    back(qq, xb)
                        else:
                            for pp in range(gn // 8):
                                xba = front(2 * pp)
                                xbb = front(2 * pp + 1)
                                rchain(pp * 8, pp * 8 + 8)
                                if upto >= 1:
                                    back(2 * pp, xba)
                                    back(2 * pp + 1, xbb)

                    def s_bias():
                        """s_i for local tiles from sq2 (fp8-consistent)."""
                        ps = blk_pool.tile([P, 3, 512], F32, tag="blk")
                        for t in range(LT):
                            nc.tensor.matmul(
                                ps[:, 0, t:t + 1],
                                lhsT=sq2[:, t * P:(t + 1) * P],
                                rhs=onescol[:], start=True, stop=True)
                        nc.vector.tensor_copy(s_loc[:], ps[:, 0, 0:LT])
                        nc.vector.tensor_scalar(
                            out=bias_sm[:], in0=s_loc[:],
                            scalar1=DELTA_SM, scalar2=None, op0=Op.add)
                        nc.vector.tensor_scalar(
                            out=bias_bg[:], in0=s_loc[:],
                            scalar1=DELTA_BG, scalar2=None, op0=Op.add)

                    def emit_esum():
                        """local e column sums (host-side mean correction)."""
                        nc.vector.tensor_reduce(
                            esum16[:],
                            ct[:, :, 0:LT * P].rearrange(
                                "p h (a c) -> p h a c", c=P),
                            axis=AX.X, op=Op.add)
                        nc.vector.tensor_reduce(
                            esum[:], esum16[:], axis=AX.X, op=Op.add)

                    def sq2sum(i0, i1):
                        """sq2 = ct0^2 + ct1^2 over 512-col chunks [i0, i1)."""
                        for i in range(i0, i1):
                            sl = slice(i * 512, (i + 1) * 512)
                            nc.gpsimd.tensor_tensor(
                                out=sq2[:, sl], in0=ct[:, 0, sl],
                                in1=ct[:, 0, sl], op=Op.mult)
                            nc.gpsimd.tensor_tensor(
                                out=sqt[:, sl], in0=ct[:, 1, sl],
                                in1=ct[:, 1, sl], op=Op.mult)
                            nc.gpsimd.tensor_tensor(
                                out=sq2[:, sl], in0=sq2[:, sl],
                                in1=sqt[:, sl], op=Op.add)

                    def ones_r():
                        return onesb[:, wrot[0] % 3, :]

                    # strip d=1..31 (3968 cols, w=2) per row-tile, in 3
                    # chunks of [1536, 1536, 896]; per-512 matmul sub-blocks
                    # (fp8 DoubleRow gram + -1/2-ones s_j pass), one wide
                    # sqrt+accum per chunk.
                    CHUNKS = [(0, 1536), (1536, 1536), (3072, 896)]

                    def strip(ci, mm):
                        coff, cw = CHUNKS[ci]
                        base = (mm + 1) * P
                        ps = blk_pool.tile([P, 3, 512], F32, tag="blk")
                        flat = ps[:].rearrange("p a c -> p (a c)")
                        nsub = (cw + 511) // 512
                        for b in range(nsub):
                            w = min(512, cw - b * 512)
                            psv = ps[:, b, 0:w]
                            c0 = base + coff + b * 512
                            nc.tensor.matmul(
                                psv,
                                lhsT=ct[:, :, mm * P:(mm + 1) * P],
                                rhs=ct[:, :, c0:c0 + w],
                                perf_mode=mybir.MatmulPerfMode.DoubleRow,
                                start=True, stop=False)
                            nc.tensor.matmul(
                                psv, lhsT=ones_r(),
                                rhs=sq2[:, c0:c0 + w],
                                start=False, stop=True)
                            wrot[0] += 1
                        # PSUM holds g - s_j/2; sqrt applies scale=-2 so the
                        # argument is s_i + s_j - 2g (+ guard).  The sqrt
                        # result is only needed for its accumulator, so it
                        # overwrites its own PSUM input (cheaper access)
                        nc.scalar.activation(
                            flat[:, 0:cw], flat[:, 0:cw], Act.Sqrt,
                            bias=bias_sm[:, mm:mm + 1], scale=-2.0,
                            accum_out=acc_w2[:, mm * 3 + ci:
                                             mm * 3 + ci + 1])

                    def combo(mm):
                        """diag (w=1) and d=32 (w=1) tiles, strided rhs."""
                        ps = blk_pool.tile([P, 3, 512], F32, tag="blk")
                        pd = ps[:, 0, 0:2 * P]
                        r0 = ct0v[:, mm:mm + 33:32, :]
                        r1 = ct1v[:, mm:mm + 33:32, :]
                        rs = sq2v[:, mm:mm + 33:32, :]
                        nc.tensor.matmul(pd, lhsT=ct[:, 0, mm * P:(mm + 1) * P],
                                         rhs=r0, start=True, stop=False)
                        nc.tensor.matmul(pd, lhsT=ct[:, 1, mm * P:(mm + 1) * P],
                                         rhs=r1, start=False, stop=False)
                        nc.tensor.matmul(pd, lhsT=ones_r(), rhs=rs,
                                         start=False, stop=True)
                        wrot[0] += 1
                        nc.scalar.activation(
                            pd, pd, Act.Sqrt,
                            bias=bias_bg[:, mm:mm + 1], scale=-2.0,
                            accum_out=acc_w1[:, mm:mm + 1])

                    # ---- interleaved emission: every engine's program is in
                    # data-arrival order so phase A (load/norm/transpose),
                    # sq2sum, and the distance blocks pipeline ----
                    bisectA = stage in (10, 11, 12, 13)
                    upto = {10: 0, 11: 1, 12: 2, 13: 3}.get(stage, 3)
                    full = stage >= 4 and not bisectA
                    has2 = stage >= 2 and not bisectA
                    has3 = stage >= 3 and not bisectA
                    if stage >= 1:
                        group(0, 8, upto)
                        group(8, 8, upto)
                        if has2:
                            sq2sum(0, 4)
                        if full:
                            s_bias()
                        if full:
                            for m in range(3):
                                strip(0, m)
                        group(16, 16, upto)
                        if has2:
                            sq2sum(4, 8)
                        group(32, 8, upto)
                        if full:
                            for m in range(3, LT):
                                strip(0, m)
                        if has2:
                            sq2sum(8, 10)
                        if full:
                            for m in range(LT):
                                strip(1, m)
                        if has3:
                            emit_esum()
                        if full:
                            for m in range(LT):
                                strip(2, m)
                            for m in range(LT):
                                combo(m)

                    # ---- Phase E: final reductions and output ----
                    nc.vector.tensor_reduce(
                        out_sb[:, 0:1], s_loc[:], axis=AX.X, op=Op.add)
                    nc.vector.tensor_reduce(
                        out_sb[:, 1:2], acc_w1[:], axis=AX.X, op=Op.add)
                    nc.vector.tensor_reduce(
                        out_sb[:, 2:3], acc_w2[:], axis=AX.X, op=Op.add)
                    nc.vector.memset(out_sb[:, 3:4], 0.0)
                    nc.vector.tensor_copy(out_sb[:, 4:6], esum[:])
                    nc.vector.tensor_reduce(
                        out_sb[:, 6:7],
                        acc_w2[:].rearrange("p (m c) -> p m c", c=3)[:, :, 2],
                        axis=AX.X, op=Op.add)
                    nc.vector.memset(out_sb[:, 7:8], 0.0)
                    nc.sync.dma_start(out=out, in_=out_sb[:])

    nc.compile()
    return nc


def kernel(embeddings: np.ndarray) -> np.ndarray:
    from concourse.bass_utils import run_bass_kernel_spmd

    X = np.ascontiguousarray(np.asarray(embeddings, dtype=np.float32))
    assert X.shape == (N, D)

    if "nc" not in _CACHE:
        _CACHE["nc"] = _build()
    nc = _CACHE["nc"]

    in_maps = [
        {"x": np.ascontiguousarray(np.roll(X, -k * LT * P, axis=0)[:NU])}
        for k in range(NCORES)
    ]
    res = run_bass_kernel_spmd(nc, in_maps, core_ids=list(range(NCORES)))

    s_sum = 0.0
    w1 = 0.0
    w2 = 0.0
    ecols = np.zeros(2 * P, dtype=np.float64)
    for k in range(NCORES):
        o = res.results[k]["out"]
        s_sum += float(o[:, 0].sum(dtype=np.float64))
        w1 += float(o[:, 1].sum(dtype=np.float64))
        w2 += float(o[:, 2].sum(dtype=np.float64))
        ecols += o[:, 4:6].astype(np.float64).T.reshape(-1)

    mu_sq = float(np.dot(ecols, ecols)) / (float(N) * float(N))
    dist_sum = w1 + 2.0 * w2
    mean_distance = dist_sum / (float(N) * float(N))
    diag_var_mean = (s_sum - float(N) * mu_sq) / float(D)
    loss = 1.0 / diag_var_mean + np.log(mean_distance)
    return np.float32(loss)



# revision 16
# speedup vs baseline: 7632.2849x; 7632.2849x over previous
"""
Trainium2 Bass kernel for EnhancedIsotropyMaximizationLoss.

loss = 1/diag_var_mean + log(mean(pairwise_L2_distance(c)))
where c = row-L2-normalized embeddings, centered by the column mean.

Key algebra:
  * distances are translation invariant -> no centering on device; the
    variance term is corrected on host: sum(c*c) = sum(e*e) - N*|mu|^2.
  * rows are unit-normalized, so s_i = |e_i|^2 == 1 exactly (the
    reference's +eps shifts s_i by ~1e-7, far below the error budget).
    Hence sq_dist = 2 - 2*g_ij with a CONSTANT guard: no s_j broadcast,
    no transposed square-sums, and diag_var_mean's sum(e*e) = N on host.
  * e is scaled by 16 before fp8 quantization (components land in
    fp8e4m3's normal range); PSUM = 256*g and the activation applies
    sqrt(-2/256 * psum + 2 + guard) with a fused accumulator.

Distribution (8 cores, no collectives): circulant decomposition of the
64x64 grid of 128-row tiles; core k's input is rotated by k*1024 rows
and covers, for each local row-tile m in 0..7: diag tile (w=1), d=1..31
(w=2), d=32 (w=1).  Only the first 40 row-tiles (2.5 MB bf16) are
loaded per core.  Tiles are strided row-subsets (row = p*8 + a within a
1024-row block) so each DMA lands >= 2 KB contiguous per partition; the
pair coverage is permutation invariant.

Engine layout: sqrt runs ONLY on ACT (Sqrt+accum; pow is not
implemented on DVE/Pool silicon), so everything else is kept off it:
all DMA kicks on SP; Pool does the norm post-math and the e-scales; DVE
does bn_stats, 1/(nrm) reciprocals, the late PSUM->fp8 drains and esum;
PE does bf16 transposes and the fp8 DoubleRow gram.  ACT's pre-sqrt
idle window absorbs the early drains and the per-chunk nrm sqrts (Copy
shares the sqrt act-table set, so there are no table reloads).  The
first 16 tiles load in 4-tile chunks to shorten the pipeline fill.
"""

import sys

if "/opt/trn_rl_repo" not in sys.path:
    sys.path.insert(0, "/opt/trn_rl_repo")

import numpy as np

N, D, P = 8192, 256, 128
NT = N // P          # 64 row tiles
NCORES = 8
LT = NT // NCORES    # 8 local row tiles per core
NTU = LT + 32        # 40 row tiles actually used per core
NU = NTU * P         # 5120 rows loaded per core
SC = 16.0            # fp8 pre-scale; PSUM gram = SC^2 * g
# sqrt guards: the combo's diag entries need 2+delta-2*s_hat >= 0 where
# s_hat = |fp8(16e)|^2/256 deviates from 1 by up to ~0.022 (0.0425
# needed); off-diagonal sq_dists are >= ~1.4.
DELTA_BG = 6e-2
DELTA_SM = 1e-4

# (tiles_start, tiles_count) per DMA chunk: 4-tile chunks first for a
# short pipeline fill, 8-tile after
CHUNK_TILES = [(0, 4), (4, 4), (8, 4), (12, 4), (16, 8), (24, 8), (32, 8)]

_CACHE = {}


def _build(stage=5, reps=1):
    import concourse.bacc as bacc
    import concourse.tile as tile
    from concourse import masks, mybir

    Op = mybir.AluOpType
    Act = mybir.ActivationFunctionType
    F32 = mybir.dt.float32
    BF16 = mybir.dt.bfloat16
    FP8 = mybir.dt.float8e4
    AX = mybir.AxisListType

    nc = bacc.Bacc("TRN2", target_bir_lowering=False, debug=False)
    x = nc.dram_tensor("x", [NU, D], BF16, kind="ExternalInput").ap()
    out = nc.dram_tensor("out", [P, 8], F32, kind="ExternalOutput").ap()

    # strip chunks per local row-tile: columns (m+1)*P .. (m+32)*P
    CHUNKS = [(0, 1536), (1536, 1536), (3072, 896)]

    with tile.TileContext(nc) as tc:
        from contextlib import ExitStack

        ctx = ExitStack()
        with ctx:
            singles = ctx.enter_context(tc.tile_pool(name="singles", bufs=1))
            ct = singles.tile([P, 2, NU], FP8, tag="ct")
            identb = singles.tile([P, P], BF16, tag="identb")
            bias_sm = singles.tile([P, 1], F32, tag="bias_sm")
            bias_bg = singles.tile([P, 1], F32, tag="bias_bg")
            acc_w1 = singles.tile([P, 2], F32, tag="acc_w1")
            acc_w2 = singles.tile([P, 3 * LT], F32, tag="acc_w2")
            esum16 = singles.tile([P, 2, LT], F32, tag="esum16")
            out_sb = singles.tile([P, 8], F32, tag="out_sb")

            nc.gpsimd.memset(bias_sm[:], 2.0 + DELTA_SM)
            nc.gpsimd.memset(bias_bg[:], 2.0 + DELTA_BG)
            nc.gpsimd.memset(acc_w1[:], 0.0)
            nc.gpsimd.memset(acc_w2[:], 0.0)
            nc.gpsimd.memset(out_sb[:], 0.0)
            masks.make_identity(nc, identb[:])
            # preload the sqrt act table; Copy shares this set so it is
            # the only table load in the whole program
            warm = singles.tile([P, 1], F32, tag="warm")
            nc.scalar.activation(warm[:], bias_sm[:], Act.Sqrt)

            for _rep in range(reps):
                with (
                    tc.tile_pool(name="xa", bufs=3) as xa_pool,
                    tc.tile_pool(name="ssg", bufs=3) as ssg_pool,
                    tc.tile_pool(name="eb", bufs=3) as eb_pool,
                    tc.tile_pool(name="ptp", bufs=2, space="PSUM") as pt_pool,
                    tc.tile_pool(name="blk", bufs=2, space="PSUM") as blk_pool,
                ):
                    xbs = {}

                    def front(c):
                        """load chunk c's tiles, row norms, r = 16/nrm."""
                        t0, tn = CHUNK_TILES[c]
                        xb = xa_pool.tile([P, tn, D], BF16, tag=f"xb{tn}")
                        xbs[c] = xb
                        nc.sync.dma_start(
                            out=xb[:],
                            in_=x[t0 * P:(t0 + tn) * P, :].rearrange(
                                "(p a) d -> p a d", a=tn))
                        bng = ssg_pool.tile([P, 8, 6], F32, tag="bng")
                        for j in range(tn):
                            nc.vector.bn_stats(bng[:, j, :], xb[:, j, :])
                        # sumsq = M2_e + M2_o + (D/2)*(mean_e^2 + mean_o^2)
                        ta = ssg_pool.tile([P, 8], F32, tag="ta")
                        tb = ssg_pool.tile([P, 8], F32, tag="tb")
                        nrm = ssg_pool.tile([P, 8], F32, tag="nrm")
                        r_g = ssg_pool.tile([P, 8], F32, tag="rg")
                        s = (slice(None), slice(0, tn))
                        nc.gpsimd.tensor_tensor(
                            out=ta[s], in0=bng[:, 0:tn, 1],
                            in1=bng[:, 0:tn, 1], op=Op.mult)
                        nc.gpsimd.tensor_tensor(
                            out=tb[s], in0=bng[:, 0:tn, 4],
                            in1=bng[:, 0:tn, 4], op=Op.mult)
                        nc.gpsimd.tensor_tensor(
                            out=ta[s], in0=ta[s], in1=tb[s], op=Op.add)
                        nc.gpsimd.tensor_tensor(
                            out=nrm[s], in0=bng[:, 0:tn, 2],
                            in1=bng[:, 0:tn, 5], op=Op.add)
                        nc.gpsimd.tensor_scalar(
                            out=ta[s], in0=ta[s], scalar1=float(D // 2),
                            scalar2=None, op0=Op.mult)
                        nc.gpsimd.tensor_tensor(
                            out=nrm[s], in0=nrm[s], in1=ta[s], op=Op.add)
                        # nrm = sqrt(sumsq)/16 on ACT (same table set as the
                        # distance sqrts); r = 1/nrm = 16*rsqrt(sumsq) on DVE
                        nc.scalar.activation(nrm[s], nrm[s], Act.Sqrt,
                                             scale=1.0 / (SC * SC))
                        nc.vector.reciprocal(r_g[s], nrm[s])
                        return r_g

                    def back(c, r_g):
                        """scale, transpose, drain chunk c into ct."""
                        xb = xbs.pop(c)
                        t0, tn = CHUNK_TILES[c]
                        if stage < 1:
                            return
                        for h in range(tn // 4):
                            q0 = h * 4
                            ebt = eb_pool.tile([P, 4, D], BF16, tag="ebt")
                            nc.gpsimd.tensor_tensor(
                                out=ebt[:], in0=xb[:, q0:q0 + 4, :],
                                in1=r_g[:, q0:q0 + 4].rearrange(
                                    "p a -> p a ()").to_broadcast([P, 4, D]),
                                op=Op.mult)
                            if stage < 2:
                                continue
                            pt = pt_pool.tile([P, 2, 4, P], BF16, tag="pt")
                            for j in range(4):
                                nc.tensor.transpose(
                                    pt[:, 0, j, :], ebt[:, j, 0:P], identb[:])
                                nc.tensor.transpose(
                                    pt[:, 1, j, :], ebt[:, j, P:2 * P],
                                    identb[:])
                            if stage < 3:
                                continue
                            c0 = (t0 + q0) * P
                            if c < 4:
                                # ACT's pre-sqrt idle window absorbs the
                                # fill-critical early drains
                                nc.scalar.activation(
                                    ct[:, :, c0:c0 + 4 * P],
                                    pt[:].rearrange("p h a d -> p h (a d)"),
                                    Act.Copy)
                            else:
                                nc.vector.tensor_copy(
                                    ct[:, :, c0:c0 + 4 * P],
                                    pt[:].rearrange("p h a d -> p h (a d)"))

                    def strip(ci, mm):
                        """distance sqrt-sum for row-tile mm, col chunk ci."""
                        coff, cw = CHUNKS[ci]
                        base = (mm + 1) * P
                        ps = blk_pool.tile([P, 3, 512], F32, tag="blk")
                        flat = ps[:].rearrange("p a c -> p (a c)")
                        for b in range((cw + 511) // 512):
                            w = min(512, cw - b * 512)
                            c0 = base + coff + b * 512
                            nc.tensor.matmul(
                                ps[:, b, 0:w],
                                lhsT=ct[:, :, mm * P:(mm + 1) * P],
                                rhs=ct[:, :, c0:c0 + w],
                                perf_mode=mybir.MatmulPerfMode.DoubleRow,
                                start=True, stop=True)
                        slot = (slice(None), slice(mm * 3 + ci, mm * 3 + ci + 1))
                        nc.scalar.activation(
                            flat[:, 0:cw], flat[:, 0:cw], Act.Sqrt,
                            bias=bias_sm[:], scale=-2.0 / (SC * SC),
                            accum_out=acc_w2[slot])

                    def combos(group):
                        """diag (w=1) and d=32 (w=1) tiles; group 0 covers
                        m=0..5 in one activation, group 1 covers m=6..7."""
                        ms = range(0, 6) if group == 0 else range(6, LT)
                        nm = len(list(ms))
                        ps = blk_pool.tile([P, 3, 512], F32, tag="blk")
                        flat = ps[:].rearrange("p a c -> p (a c)")
                        for i, mm in enumerate(ms):
                            pd = flat[:, i * 256:(i + 1) * 256]
                            for u in range(2):
                                nc.tensor.matmul(
                                    pd[:, u * P:(u + 1) * P],
                                    lhsT=ct[:, :, mm * P:(mm + 1) * P],
                                    rhs=ct[:, :, (mm + 32 * u) * P:
                                           (mm + 32 * u + 1) * P],
                                    perf_mode=mybir.MatmulPerfMode.DoubleRow,
                                    start=True, stop=True)
                        nc.scalar.activation(
                            flat[:, 0:nm * 256], flat[:, 0:nm * 256],
                            Act.Sqrt, bias=bias_bg[:], scale=-2.0 / (SC * SC),
                            accum_out=acc_w1[:, group:group + 1])

                    def emit_esum():
                        """column sums of local e rows (for host mu); DVE is
                        idle once its drains are done."""
                        nc.vector.tensor_reduce(
                            esum16[:],
                            ct[:, :, 0:LT * P].rearrange(
                                "p h (a c) -> p h a c", c=P),
                            axis=AX.X, op=Op.add)
                        nc.vector.tensor_reduce(
                            out_sb[:, 4:6], esum16[:], axis=AX.X, op=Op.add)

                    full = stage >= 4

                    rs = {}
                    rs[0] = front(0)
                    rs[1] = front(1)
                    rs[2] = front(2)
                    rs[3] = front(3)
                    back(0, rs[0])
                    back(1, rs[1])
                    back(2, rs[2])
                    back(3, rs[3])
                    rs[4] = front(4)
                    if full:
                        for m in range(3):
                            strip(0, m)
                    back(4, rs[4])
                    rs[5] = front(5)
                    if full:
                        for m in range(3, LT):
                            strip(0, m)
                    back(5, rs[5])
                    rs[6] = front(6)
                    if full:
                        for m in range(3):
                            strip(1, m)
                    back(6, rs[6])
                    if full:
                        for m in range(3, LT):
                            strip(1, m)
                    if stage >= 3:
                        emit_esum()
                    if full:
                        for m in range(LT):
                            strip(2, m)
                        combos(0)
                        combos(1)

                    # ---- final reductions and output ----
                    nc.vector.tensor_reduce(
                        out_sb[:, 0:1], acc_w1[:], axis=AX.X, op=Op.add)
                    nc.vector.tensor_reduce(
                        out_sb[:, 1:2], acc_w2[:], axis=AX.X, op=Op.add)
                    nc.sync.dma_start(out=out, in_=out_sb[:])

    nc.compile()
    return nc


def _prep_inputs(X):
    import ml_dtypes

    Xb = np.asarray(X, dtype=ml_dtypes.bfloat16)
    return [
        {"x": np.ascontiguousarray(np.roll(Xb, -k * LT * P, axis=0)[:NU])}
        for k in range(NCORES)
    ]


def kernel(embeddings: np.ndarray) -> np.ndarray:
    from concourse.bass_utils import run_bass_kernel_spmd

    X = np.ascontiguousarray(np.asarray(embeddings, dtype=np.float32))
    assert X.shape == (N, D)

    if "nc" not in _CACHE:
        _CACHE["nc"] = _build()
    nc = _CACHE["nc"]

    in_maps = _prep_inputs(X)
    res = run_bass_kernel_spmd(nc, in_maps, core_ids=list(range(NCORES)))

    return _combine([res.results[k]["out"] for k in range(NCORES)])


def _combine(outs) -> np.ndarray:
    """Host-side reduction of the 8 per-core [P, 8] output tiles."""
    w1 = 0.0
    w2 = 0.0
    ecols = np.zeros(2 * P, dtype=np.float64)
    for o in outs:
        w1 += float(o[:, 0].sum(dtype=np.float64))
        w2 += float(o[:, 1].sum(dtype=np.float64))
        ecols += o[:, 4:6].astype(np.float64).T.reshape(-1)

    ec = ecols / SC
    mu_sq = float(np.dot(ec, ec)) / (float(N) * float(N))
    dist_sum = w1 + 2.0 * w2
    mean_distance = dist_sum / (float(N) * float(N))
    diag_var_mean = (float(N) - float(N) * mu_sq) / float(D)
    loss = 1.0 / diag_var_mean + np.log(mean_distance)
    return np.float32(loss)
